# revision 13
# baseline (speedup 1.0000x reference)
"""Multi-head attention (B=2, S=2048, D=768, H=12) on 8 NeuronCores.

Sharding: data-parallel over batch (2) x tensor-parallel over heads (4 groups
of 3 heads) = 8 cores. Each core computes its 3 heads' Q/K/V projections,
attention, and a partial output projection; the host sums the 4 per-batch
partials and adds the output bias plus the V-bias contribution b_v @ w_o.T
(V bias commutes through softmax-normalized attention into a constant).

Bias algebra: softmax over keys is shift-invariant, so the K bias drops
exactly. Q bias is added via a rank-1 ones matmul; V/O biases move to the
host.

Per-core layout (E = 192 = 3 heads x 64):
  Inputs arrive pre-packed contiguous so each DMA trigger moves full 4KB+
  lines: x [128, 6*2048] bf16, merged QK weights [128, 6*384] bf16
  (m-tiles [q0|q1], [k0|q2], [k2|k1] so every head's q/k share a partition
  range), wv [128, 6*192] bf16, wo [128, 2*768] f32, bq [1, 384] bf16.
  Projections: 3 full-width merged QK m-tiles, then V in natural [sk, e]
  layout with a ones column per head (the ctx matmul then emits softmax
  denominators in PSUM row 64 for free).
  Attention: software-pipelined two iterations deep (scores of t+2 issue
  before ctx of t) so the PE never waits on exp. Exp is split per
  512-column block: ScalarE true exp on block A, VectorE one-instruction
  Schraudolph fast-exp (int16 bf16-bit-pattern trick, ~1.5% RMS) on block
  B. Heads 0,1 pair within an iteration; head 2 pairs two sq chunks.
  The second pair's iterations carry the first pair's output projection;
  the last pair's runs as the tail on freed sp slots. Out tiles stream to
  HBM per (d-chunk, sq-chunk).
  HAM: the PE clock gate counts only MATMUL activity (LDWEIGHTS do not
  register), throttles to half clock after any low-activity window, and
  recovers slowly - so the DMA lead-in and x-paced projection gaps are
  bridged with tiny dummy matmuls into a scratch PSUM tile, and the
  attention stream is kept matmul-dense by construction.
"""

import sys

sys.path.insert(0, "/opt/trn_rl_repo")

import numpy as np

B, S, D = 2, 2048, 768
H, DK = 12, 64
P = 128
HG = 3              # heads per core
E = HG * DK         # 192
KD = D // P         # 6 contraction chunks
SQC = S // 512      # 4 sq chunks
SKC = S // P        # 16 sk chunks
SCALE = 1.0 / 8.0   # 1/sqrt(dk)

# Schraudolph fast-exp constants in bf16-bit units (scale folded in):
# i16 = int(x * A + B), bitcast bf16 ~= exp(x * SCALE), ~1.5% RMS
FEXP_A = float(np.float32(SCALE * (1 << 7) / np.log(2.0)))
FEXP_B = float(np.float32(127.0 * (1 << 7) - 7.42))

_NC_CACHE = {}


def _build_bass():
    import concourse.bacc as bacc
    import concourse.tile as tile
    from concourse import mybir

    f32 = mybir.dt.float32
    f32r = mybir.dt.float32r
    bf16 = mybir.dt.bfloat16
    i16 = mybir.dt.int16
    Exp = mybir.ActivationFunctionType.Exp
    Copy = mybir.ActivationFunctionType.Copy
    MUL = mybir.AluOpType.mult
    ADD = mybir.AluOpType.add

    nc = bacc.Bacc(trn_type="TRN2", debug=False)

    xP = nc.dram_tensor("xP", [P, KD * S], bf16, kind="ExternalInput")
    wqkP = nc.dram_tensor("wqkP", [P, KD * 384], bf16, kind="ExternalInput")
    wvP = nc.dram_tensor("wvP", [P, KD * E], bf16, kind="ExternalInput")
    woP = nc.dram_tensor("woP", [P, 2 * D], f32, kind="ExternalInput")
    bqP = nc.dram_tensor("bqP", [1, 384], bf16, kind="ExternalInput")
    outT = nc.dram_tensor("outT", [D, S], f32, kind="ExternalOutput")

    outT_d = outT.ap().rearrange("(e p) s -> e p s", p=P)

    with tile.TileContext(nc) as tc:
        with tc.tile_pool(name="persist", bufs=1) as persist, \
             tc.tile_pool(name="work", bufs=2) as work, \
             tc.tile_pool(name="small", bufs=2) as small:

            # ---- persistent SBUF ----
            x_sb = persist.tile([P, KD * S], bf16, tag="x")
            wqk_sb = persist.tile([P, KD * 384], bf16, tag="wqk")
            wv_sb = persist.tile([P, KD * E], bf16, tag="wv")
            wo_sb = persist.tile([P, 2 * D], f32r, tag="wo")
            bq_sb = persist.tile([1, 384], bf16, tag="bq")
            # projection outputs (bf16): At = [q0 | q1], Bt = [k0 | q2],
            # Ct = [k2 | k1]  (rows 0-63 | 64-127)
            At = persist.tile([P, S], bf16, tag="At")
            Bt = persist.tile([P, S], bf16, tag="Bt")
            Ct = persist.tile([P, S], bf16, tag="Ct")
            qh2 = persist.tile([64, S], bf16, tag="qh2")  # q2 at parts 0-63
            v_sb = [persist.tile([P, HG, 65], bf16, tag=f"v{i}", name=f"v{i}")
                    for i in range(SKC)]
            ctx_a = persist.tile([P, S], f32r, tag="ctx_a")   # h0 | h1
            ctx_b = persist.tile([64, S], f32r, tag="ctx_b")  # h2
            ones_bf = persist.tile([1, 512], bf16, tag="ones_bf")
            warmw = persist.tile([P, 64], bf16, tag="warmw")
            warm_in = small.tile([1, 8], f32, tag="warm_in")
            et_warm = small.tile([1, 8], f32, tag="warm_out")

            nc.vector.memset(ones_bf[:], 1.0)
            nc.vector.memset(warmw[:], 0.5)
            nc.vector.memset(warm_in[:], 0.0)
            ones_f32 = small.tile([1, 64], f32, tag="ones_f32")
            nc.vector.memset(ones_f32[:], 1.0)
            ones_col = persist.tile([1, 64], f32r, tag="ones_col")
            nc.vector.tensor_copy(ones_col[:], ones_f32[:])
            # preload the exp activation table (first ACTIVATE to a new
            # table set costs ~2.7us)
            nc.scalar.activation(et_warm[:], warm_in[:], Exp, scale=1.0)

            # ---- input DMA: few triggers, contiguous 4KB+ lines ----
            nc.sync.dma_start(out=bq_sb[:], in_=bqP.ap())
            nc.sync.dma_start(out=wqk_sb[:], in_=wqkP.ap())
            for d in range(KD):
                nc.sync.dma_start(out=x_sb[:, d * S:(d + 1) * S],
                                  in_=xP.ap()[:, d * S:(d + 1) * S])
            nc.sync.dma_start(out=wv_sb[:], in_=wvP.ap())
            nc.sync.dma_start(out=wo_sb[:], in_=woP.ap().bitcast(f32r))

            # ================= QKV projections =================
            with tc.tile_pool(name="proj_ps", bufs=6, space="PSUM") as proj_ps:
                wps = proj_ps.tile([64, 64], f32, tag="warm", bufs=1)

                def mm_fill(n):
                    # tiny always-ready matmuls into a scratch PSUM tile;
                    # the HAM clock gate counts only matmul activity, so
                    # these keep/get the PE warm across DMA-paced gaps
                    for _ in range(n):
                        nc.tensor.matmul(wps[:], warmw[:, 0:64],
                                         warmw[:, 0:64], start=True, stop=True)

                # warm the clock gate while the x/weight DMA lands
                mm_fill(48)

                # merged QK: m-outer, d-inner (x-DMA-paced on m0)
                for m in range(3):
                    ps = [proj_ps.tile([P, 512], f32, tag="qk", bufs=4,
                                       name=f"qk_{m}_{c}") for c in range(SQC)]
                    for d in range(KD):
                        w_sl = wqk_sb[:, d * 384 + m * P: d * 384 + (m + 1) * P]
                        for c in range(SQC):
                            nc.tensor.matmul(
                                ps[c][:], w_sl,
                                x_sb[:, d * S + c * 512: d * S + (c + 1) * 512],
                                start=(d == 0),
                                stop=(d == KD - 1 and m == 2),
                            )
                        if m == 0 and d < KD - 1:
                            mm_fill(10)  # bridge the x_{d+1} DMA wait
                    dest = (At, Bt, Ct)[m]
                    for c in range(SQC):
                        if m < 2:  # rank-1 Q-bias row (zeros over k columns)
                            nc.tensor.matmul(
                                ps[c][:], bq_sb[0:1, m * P:(m + 1) * P],
                                ones_bf[0:1, :], start=False, stop=True,
                            )
                        nc.vector.tensor_copy(
                            dest[:, c * 512:(c + 1) * 512], ps[c][:])
                # move q2 (Bt rows 64-127) down to partitions 0-63 so head 2's
                # scores contract matching partition ranges
                nc.sync.dma_start(out=qh2[:], in_=Bt[64:P, :])

                # V projection, natural [sk, e] layout
                for i in range(SKC):
                    vps = proj_ps.tile([P, E], f32, tag="vps", bufs=2,
                                       name=f"vps_{i}")
                    for d in range(KD):
                        nc.tensor.matmul(
                            vps[:],
                            x_sb[:, d * S + i * P: d * S + (i + 1) * P],
                            wv_sb[:, d * E:(d + 1) * E],
                            start=(d == 0), stop=(d == KD - 1),
                        )
                    nc.vector.tensor_copy(
                        v_sb[i][:, :, 0:64],
                        vps[:].rearrange("p (h d) -> p h d", h=HG))
                    nc.vector.memset(v_sb[i][:, :, 64:65], 1.0)

            def make_normalize(psum_pool, cps_bufs):
                def normalize(cps, ctx_dst, key, eng):
                    # The PSUM reads (den row 64 + cu rows 0-63) must be
                    # emitted inline, BEFORE the next chunk's ctx matmul
                    # recycles this cps slot — the dependency tracker only
                    # sees already-emitted reads. Downstream SBUF pieces
                    # (reciprocal, f32r cast, rank-1 PE broadcast matmul,
                    # final DVE scale) return staggered so no engine FIFO
                    # backs up. `eng` alternates the inline copies between
                    # ScalarE and DVE per head.
                    den = small.tile([1, 512], f32, tag="den")
                    cu = work.tile([64, 512], f32r, tag="cu", bufs=3)
                    if eng == 0:
                        nc.scalar.activation(den[:], cps[64:65, :], Copy)
                        nc.scalar.activation(cu[:], cps[0:64, :], Copy)
                    else:
                        nc.vector.tensor_copy(den[:], cps[64:65, :])
                        nc.vector.tensor_copy(cu[:], cps[0:64, :])
                    r = small.tile([1, 512], f32, tag="r")
                    r2 = small.tile([1, 512], f32r, tag="r2")
                    rb = psum_pool.tile([65, 512], f32, tag="cps",
                                        bufs=cps_bufs, name=f"rb_{key}")
                    return [
                        (1, lambda: nc.vector.reciprocal_approx_fast(
                            r[:], den[:])),
                        (2, lambda: nc.vector.tensor_copy(r2[:], r[:])),
                        (3, lambda: nc.tensor.matmul(
                            rb[0:64, :], ones_col[:], r2[:],
                            start=True, stop=True)),
                        (4, lambda: nc.vector.tensor_mul(
                            ctx_dst, cu[:], rb[0:64, :])),
                    ]
                return normalize

            # ================= attention: heads 0,1 =================
            # per (sq chunk c, sk chunk i): two score matmuls [128, 512]
            # (h0 block A, h1 block B), ScalarE exp on A / DVE fast-exp on
            # B, two ctx accumulations [65, 512]. Pipelined 2 deep.
            with tc.tile_pool(name="p1", bufs=2, space="PSUM") as p1:
                normalize = make_normalize(p1, 4)
                steps = [(c, i) for c in range(SQC) for i in range(SKC)]
                ets, cps = {}, {}
                pending = {}

                def push_pieces(t, pieces):
                    for off, fn in pieces:
                        pending.setdefault(t + off, []).append(fn)

                def run_pieces(t):
                    for fn in pending.pop(t, ()):
                        fn()

                def emit_scores01(t):
                    c, i = steps[t]
                    sp_a = p1.tile([P, 512], f32, tag="sp", bufs=4,
                                   name=f"spA_{t}")
                    if i == 0:
                        # boundary HAM insurance: dummy matmuls, overwritten
                        # by the real scores matmul via start=True
                        for _ in range(2):
                            nc.tensor.matmul(sp_a[0:64, 0:128], warmw[:, :],
                                             x_sb[:, 0:128],
                                             start=True, stop=True)
                    nc.tensor.matmul(
                        sp_a[:], Bt[0:64, i * P:(i + 1) * P],
                        At[0:64, c * 512:(c + 1) * 512], start=True, stop=True)
                    sp_b = p1.tile([P, 512], f32, tag="sp", bufs=4,
                                   name=f"spB_{t}")
                    nc.tensor.matmul(
                        sp_b[:], Ct[64:P, i * P:(i + 1) * P],
                        At[64:P, c * 512:(c + 1) * 512], start=True, stop=True)
                    et_a = work.tile([P, 512], bf16, tag="et", bufs=6,
                                     name=f"etA_{t}")
                    nc.scalar.activation(et_a[:], sp_a[:], Exp, scale=SCALE)
                    et_b = work.tile([P, 512], bf16, tag="et", bufs=6,
                                     name=f"etB_{t}")
                    nc.vector.tensor_scalar(et_b[:].bitcast(i16), sp_b[:],
                                            FEXP_A, FEXP_B, MUL, ADD)
                    ets[t] = (et_a, et_b)

                def emit_ctx01(t):
                    c, i = steps[t]
                    if i == 0:
                        cps[c] = [p1.tile([65, 512], f32, tag="cps", bufs=4,
                                          name=f"cps01_{c}_{j}")
                                  for j in range(2)]
                    et_a, et_b = ets.pop(t)
                    nc.tensor.matmul(cps[c][0][:], v_sb[i][:, 0, :], et_a[:],
                                     start=(i == 0), stop=(i == SKC - 1))
                    nc.tensor.matmul(cps[c][1][:], v_sb[i][:, 1, :], et_b[:],
                                     start=(i == 0), stop=(i == SKC - 1))
                    if i == SKC - 1:
                        for j in range(2):
                            push_pieces(t + 1 + 2 * j, normalize(
                                cps[c][j],
                                ctx_a[j * 64:(j + 1) * 64,
                                      c * 512:(c + 1) * 512],
                                f"a{c}_{j}", j))

                DEPTH = 2
                for t in range(len(steps) + DEPTH + 8):
                    if t < len(steps):
                        emit_scores01(t)
                    if DEPTH <= t < len(steps) + DEPTH:
                        emit_ctx01(t - DEPTH)
                    run_pieces(t - DEPTH)

            # ================= head 2 + output projection =================
            # per (sq-chunk pair g, sk chunk i): two score matmuls (chunk
            # 2g block A, chunk 2g+1 block B) sharing one stationary kt,
            # split exp, two ctx accumulations. Pair 1's iterations carry
            # pair 0's output projection; pair 1's runs as the tail on the
            # freed sp slots.
            with tc.tile_pool(name="p2", bufs=2, space="PSUM") as p2:
                normalize = make_normalize(p2, 3)
                ets2, cps2 = {}, {}
                pending2 = {}

                def push_pieces2(t, pieces):
                    for off, fn in pieces:
                        pending2.setdefault(t + off, []).append(fn)

                def run_pieces2(t):
                    for fn in pending2.pop(t, ()):
                        fn()
                # weave slots for pair 0's output projection inside pair
                # 1's iterations: start at i=5 so the staggered normalize
                # multiplies (emitted up to 4 iterations past the boundary)
                # land first. Chunk 0 -> i 5..10, chunk 1 -> i 11..15
                # (e=0..4); chunk 1's e=5 joins the tail.
                weave_slots = list(range(5, SKC))

                def emit_scores2(t):
                    g, i = divmod(t, SKC)
                    pair = []
                    for j in range(2):
                        c = 2 * g + j
                        sp = p2.tile([P, 512], f32, tag="sp", bufs=4,
                                     name=f"sp2_{t}_{j}")
                        if i == 0 and j == 0:
                            for _ in range(2):
                                nc.tensor.matmul(sp[0:64, 0:128], warmw[:, :],
                                                 x_sb[:, 0:128],
                                                 start=True, stop=True)
                        nc.tensor.matmul(
                            sp[:], Ct[0:64, i * P:(i + 1) * P],
                            qh2[:, c * 512:(c + 1) * 512],
                            start=True, stop=True)
                        pair.append(sp)
                    et_a = work.tile([P, 512], bf16, tag="et", bufs=6,
                                     name=f"et2A_{t}")
                    nc.scalar.activation(et_a[:], pair[0][:], Exp, scale=SCALE)
                    et_b = work.tile([P, 512], bf16, tag="et", bufs=6,
                                     name=f"et2B_{t}")
                    nc.vector.tensor_scalar(et_b[:].bitcast(i16), pair[1][:],
                                            FEXP_A, FEXP_B, MUL, ADD)
                    ets2[t] = (et_a, et_b)

                def emit_ctx2(t):
                    g, i = divmod(t, SKC)
                    if i == 0:
                        cps2[g] = [p2.tile([65, 512], f32, tag="cps", bufs=3,
                                           name=f"cps2_{g}_{j}")
                                   for j in range(2)]
                    et_a, et_b = ets2.pop(t)
                    for j in range(2):
                        nc.tensor.matmul(
                            cps2[g][j][:], v_sb[i][:, 2, :],
                            (et_a, et_b)[j][:],
                            start=(i == 0), stop=(i == SKC - 1))
                    if i == SKC - 1:
                        for j in range(2):
                            push_pieces2(t + 1 + 2 * j, normalize(
                                cps2[g][j],
                                ctx_b[:, (2 * g + j) * 512:
                                      (2 * g + j + 1) * 512],
                                f"b{g}_{j}", j))

                def emit_outproj(c, e, op_tag):
                    op = p2.tile([P, 512], f32, tag=op_tag,
                                 bufs=(1 if op_tag == "op" else 4),
                                 name=f"op_{c}_{e}")
                    nc.tensor.matmul(
                        op[:], wo_sb[:, e * P:(e + 1) * P],
                        ctx_a[:, c * 512:(c + 1) * 512],
                        start=True, stop=False)
                    nc.tensor.matmul(
                        op[:], wo_sb[0:64, D + e * P: D + (e + 1) * P],
                        ctx_b[:, c * 512:(c + 1) * 512],
                        start=False, stop=True)
                    o = work.tile([P, 512], f32, tag="o", bufs=4,
                                  name=f"o_{c}_{e}")
                    if e % 2 == 0:
                        nc.vector.tensor_copy(o[:], op[:])
                    else:
                        nc.scalar.activation(o[:], op[:], Copy)
                    nc.sync.dma_start(
                        out=outT_d[e][:, c * 512:(c + 1) * 512], in_=o[:])

                DEPTH = 2
                n2 = 2 * SKC
                for t in range(n2 + DEPTH + 8):
                    if t < n2:
                        emit_scores2(t)
                    if DEPTH <= t < n2 + DEPTH:
                        tt = t - DEPTH
                        emit_ctx2(tt)
                        g, i = divmod(tt, SKC)
                        if g == 1 and i in weave_slots:
                            k = weave_slots.index(i)
                            emit_outproj(k // KD, k % KD, "op")
                    run_pieces2(t - DEPTH)
                # tail: chunk 1's last e-chunk plus the final pair's
                # output projection, on freed sp slots
                emit_outproj(1, 5, "sp")
                for c in (2, 3):
                    for e in range(KD):
                        emit_outproj(c, e, "sp")

    nc.finalize()
    return nc


def _get_nc():
    if "nc" not in _NC_CACHE:
        _NC_CACHE["nc"] = _build_bass()
    return _NC_CACHE["nc"]


def _core_inputs(c, x, w_q, b_q, w_k, w_v, w_o):
    import ml_dtypes
    bf = ml_dtypes.bfloat16
    b, g = divmod(c, 4)
    gs = slice(g * E, (g + 1) * E)

    def pack6(a):  # [768, W] -> [128, 6*W] chunk-packed
        w = a.shape[1]
        return np.ascontiguousarray(
            a.reshape(KD, P, w).transpose(1, 0, 2).reshape(P, KD * w))

    wqT = np.ascontiguousarray(w_q[gs, :].T)  # [768, 192]
    wkT = np.ascontiguousarray(w_k[gs, :].T)
    # m-tiles: [q0|q1], [k0|q2], [k2|k1]
    wqk = np.concatenate([wqT[:, 0:128], wkT[:, 0:64], wqT[:, 128:192],
                          wkT[:, 128:192], wkT[:, 64:128]], axis=1)
    bq = b_q[gs]
    bqp = np.zeros((1, 384), np.float32)
    bqp[0, 0:128] = bq[0:128]
    bqp[0, 192:256] = bq[128:192]
    woT = np.ascontiguousarray(w_o[:, gs].T)  # [192, 768]
    wop = np.zeros((P, 2 * D), np.float32)
    wop[:, 0:D] = woT[0:P]
    wop[0:64, D:] = woT[P:E]
    return {
        "xP": pack6(np.ascontiguousarray(x[b].T)).astype(bf),
        "wqkP": pack6(wqk).astype(bf),
        "wvP": pack6(np.ascontiguousarray(w_v[gs, :].T)).astype(bf),
        "woP": wop,
        "bqP": bqp.astype(bf),
    }


def kernel(x, w_q, b_q, w_k, b_k, w_v, b_v, w_o, b_o, _trace=False):
    from concourse.bass_utils import run_bass_kernel_spmd

    x = np.asarray(x, np.float32)
    w_q, b_q, w_k = (np.asarray(a, np.float32) for a in (w_q, b_q, w_k))
    w_v, w_o = np.asarray(w_v, np.float32), np.asarray(w_o, np.float32)
    b_v, b_o = np.asarray(b_v, np.float32), np.asarray(b_o, np.float32)

    nc = _get_nc()
    in_maps = [_core_inputs(c, x, w_q, b_q, w_k, w_v, w_o) for c in range(8)]
    res = run_bass_kernel_spmd(nc, in_maps, core_ids=list(range(8)),
                               trace=_trace)

    out = np.zeros((B, S, D), np.float32)
    for c in range(8):
        out[c // 4] += res.results[c]["outT"].T
    # K bias drops via softmax shift-invariance; V and O biases are constants
    out += b_o + b_v @ w_o.T
    if _trace:
        kernel._last_results = res
    return out


# revision 14
# speedup vs baseline: 1.1898x; 1.1898x over previous
"""Multi-head attention (B=2, S=2048, D=768, H=12) on 8 NeuronCores.

Sharding: data-parallel over batch (2) x tensor-parallel over heads (4 groups
of 3 heads) = 8 cores. Each core computes its 3 heads' Q/K/V projections,
attention, and a partial output projection; the host sums the 4 per-batch
partials and adds the output bias plus the V-bias contribution b_v @ w_o.T
(V bias commutes through softmax-normalized attention into a constant).

Bias algebra: softmax over keys is shift-invariant, so the K bias drops
exactly. Q bias is added via a rank-1 ones matmul; V/O biases move to the
host.

Per-core layout (E = 192 = 3 heads x 64):
  Inputs arrive pre-packed contiguous so each DMA trigger moves full 4KB+
  lines: x [128, 6*2048] bf16, merged QK weights [128, 6*384] bf16
  (m-tiles [q0|q1], [k0|q2], [k2|k1] so every head's q/k share a partition
  range), wv [128, 6*192] bf16, wo [128, 2*768] f32, bq [1, 384] bf16.
  Projections: 3 full-width merged QK m-tiles, then V in natural [sk, e]
  layout with a ones column per head (the ctx matmul then emits softmax
  denominators in PSUM row 64 for free).
  Attention: software-pipelined two iterations deep (scores of t+2 issue
  before ctx of t) so the PE never waits on exp. Exp is split per
  512-column block: ScalarE true exp on block A, VectorE one-instruction
  Schraudolph fast-exp (int16 bf16-bit-pattern trick, ~1.5% RMS) on block
  B. Heads 0,1 pair within an iteration; head 2 pairs two sq chunks.
  The second pair's iterations carry the first pair's output projection;
  the last pair's runs as the tail on freed sp slots. Out tiles stream to
  HBM per (d-chunk, sq-chunk).
  HAM: the PE clock gate counts only MATMUL activity (LDWEIGHTS do not
  register), throttles to half clock after any low-activity window, and
  recovers slowly - so the DMA lead-in and x-paced projection gaps are
  bridged with tiny dummy matmuls into a scratch PSUM tile, and the
  attention stream is kept matmul-dense by construction.
"""

import sys

sys.path.insert(0, "/opt/trn_rl_repo")

import numpy as np

B, S, D = 2, 2048, 768
H, DK = 12, 64
P = 128
HG = 3              # heads per core
E = HG * DK         # 192
KD = D // P         # 6 contraction chunks
SQC = S // 512      # 4 sq chunks
SKC = S // P        # 16 sk chunks
SCALE = 1.0 / 8.0   # 1/sqrt(dk)

# Schraudolph fast-exp constants in bf16-bit units (scale folded in):
# i16 = int(x * A + B), bitcast bf16 ~= exp(x * SCALE), ~1.5% RMS
FEXP_A = float(np.float32(SCALE * (1 << 7) / np.log(2.0)))
FEXP_B = float(np.float32(127.0 * (1 << 7) - 7.42))

_NC_CACHE = {}


def _build_bass():
    import concourse.bacc as bacc
    import concourse.tile as tile
    from concourse import mybir

    f32 = mybir.dt.float32
    f32r = mybir.dt.float32r
    bf16 = mybir.dt.bfloat16
    i16 = mybir.dt.int16
    Exp = mybir.ActivationFunctionType.Exp
    Copy = mybir.ActivationFunctionType.Copy
    MUL = mybir.AluOpType.mult
    ADD = mybir.AluOpType.add

    nc = bacc.Bacc(trn_type="TRN2", debug=False)

    xP = nc.dram_tensor("xP", [P, KD * S], bf16, kind="ExternalInput")
    wqkP = nc.dram_tensor("wqkP", [P, KD * 384], bf16, kind="ExternalInput")
    wvP = nc.dram_tensor("wvP", [P, KD * E], bf16, kind="ExternalInput")
    woP = nc.dram_tensor("woP", [P, 2 * D], f32, kind="ExternalInput")
    bqP = nc.dram_tensor("bqP", [1, 384], bf16, kind="ExternalInput")
    outT = nc.dram_tensor("outT", [D, S], f32, kind="ExternalOutput")

    outT_d = outT.ap().rearrange("(e p) s -> e p s", p=P)

    with tile.TileContext(nc) as tc:
        with tc.tile_pool(name="persist", bufs=1) as persist, \
             tc.tile_pool(name="work", bufs=2) as work, \
             tc.tile_pool(name="small", bufs=2) as small:

            # ---- persistent SBUF ----
            x_sb = persist.tile([P, KD * S], bf16, tag="x")
            wqk_sb = persist.tile([P, KD * 384], bf16, tag="wqk")
            wv_sb = persist.tile([P, KD * E], bf16, tag="wv")
            wo_sb = persist.tile([P, 2 * D], f32r, tag="wo")
            bq_sb = persist.tile([1, 384], bf16, tag="bq")
            # projection outputs (bf16): At = [q0 | q1], Bt = [k0 | q2],
            # Ct = [k2 | k1]  (rows 0-63 | 64-127)
            At = persist.tile([P, S], bf16, tag="At")
            Bt = persist.tile([P, S], bf16, tag="Bt")
            Ct = persist.tile([P, S], bf16, tag="Ct")
            qh2 = persist.tile([64, S], bf16, tag="qh2")  # q2 at parts 0-63
            v_sb = [persist.tile([P, HG, 65], bf16, tag=f"v{i}", name=f"v{i}")
                    for i in range(SKC)]
            ctx_a = persist.tile([P, S], f32r, tag="ctx_a")   # h0 | h1
            ctx_b = persist.tile([64, S], f32r, tag="ctx_b")  # h2
            ones_bf = persist.tile([1, 512], bf16, tag="ones_bf")
            warmw = persist.tile([P, 64], bf16, tag="warmw")
            warm_in = small.tile([1, 8], f32, tag="warm_in")
            et_warm = small.tile([1, 8], f32, tag="warm_out")

            nc.vector.memset(ones_bf[:], 1.0)
            nc.vector.memset(warmw[:], 0.5)
            nc.vector.memset(warm_in[:], 0.0)
            ones_f32 = small.tile([1, 64], f32, tag="ones_f32")
            nc.vector.memset(ones_f32[:], 1.0)
            ones_col = persist.tile([1, 64], f32r, tag="ones_col")
            nc.vector.tensor_copy(ones_col[:], ones_f32[:])
            # preload the exp activation table (first ACTIVATE to a new
            # table set costs ~2.7us)
            nc.scalar.activation(et_warm[:], warm_in[:], Exp, scale=1.0)

            # ---- input DMA: few triggers, contiguous 4KB+ lines ----
            nc.sync.dma_start(out=bq_sb[:], in_=bqP.ap())
            nc.sync.dma_start(out=wqk_sb[:], in_=wqkP.ap())
            for d in range(KD):
                nc.sync.dma_start(out=x_sb[:, d * S:(d + 1) * S],
                                  in_=xP.ap()[:, d * S:(d + 1) * S])
            nc.sync.dma_start(out=wv_sb[:], in_=wvP.ap())
            nc.sync.dma_start(out=wo_sb[:], in_=woP.ap().bitcast(f32r))

            # ================= QKV projections =================
            with tc.tile_pool(name="proj_ps", bufs=6, space="PSUM") as proj_ps:
                wps = proj_ps.tile([64, 64], f32, tag="warm", bufs=1)

                def mm_fill(n):
                    # tiny always-ready matmuls into a scratch PSUM tile;
                    # the HAM clock gate counts only matmul activity, so
                    # these keep/get the PE warm across DMA-paced gaps
                    for _ in range(n):
                        nc.tensor.matmul(wps[:], warmw[:, 0:64],
                                         warmw[:, 0:64], start=True, stop=True)

                # warm the clock gate while the x/weight DMA lands
                mm_fill(48)

                # merged QK: m-outer, d-inner (x-DMA-paced on m0)
                for m in range(3):
                    ps = [proj_ps.tile([P, 512], f32, tag="qk", bufs=4,
                                       name=f"qk_{m}_{c}") for c in range(SQC)]
                    for d in range(KD):
                        w_sl = wqk_sb[:, d * 384 + m * P: d * 384 + (m + 1) * P]
                        for c in range(SQC):
                            nc.tensor.matmul(
                                ps[c][:], w_sl,
                                x_sb[:, d * S + c * 512: d * S + (c + 1) * 512],
                                start=(d == 0),
                                stop=(d == KD - 1 and m == 2),
                            )
                        if m == 0 and d < KD - 1:
                            mm_fill(22)  # bridge the x_{d+1} DMA wait
                    dest = (At, Bt, Ct)[m]
                    for c in range(SQC):
                        if m < 2:  # rank-1 Q-bias row (zeros over k columns)
                            nc.tensor.matmul(
                                ps[c][:], bq_sb[0:1, m * P:(m + 1) * P],
                                ones_bf[0:1, :], start=False, stop=True,
                            )
                        nc.vector.tensor_copy(
                            dest[:, c * 512:(c + 1) * 512], ps[c][:])
                # move q2 (Bt rows 64-127) down to partitions 0-63 so head 2's
                # scores contract matching partition ranges
                nc.sync.dma_start(out=qh2[:], in_=Bt[64:P, :])

                # V projection, natural [sk, e] layout
                for i in range(SKC):
                    vps = proj_ps.tile([P, E], f32, tag="vps", bufs=2,
                                       name=f"vps_{i}")
                    for d in range(KD):
                        nc.tensor.matmul(
                            vps[:],
                            x_sb[:, d * S + i * P: d * S + (i + 1) * P],
                            wv_sb[:, d * E:(d + 1) * E],
                            start=(d == 0), stop=(d == KD - 1),
                        )
                    nc.vector.tensor_copy(
                        v_sb[i][:, :, 0:64],
                        vps[:].rearrange("p (h d) -> p h d", h=HG))
                    nc.vector.memset(v_sb[i][:, :, 64:65], 1.0)

            def make_normalize(psum_pool, cps_bufs):
                def normalize(cps, ctx_dst, key, eng):
                    # The PSUM reads (den row 64 + cu rows 0-63) must be
                    # emitted inline, BEFORE the next chunk's ctx matmul
                    # recycles this cps slot — the dependency tracker only
                    # sees already-emitted reads. Downstream SBUF pieces
                    # (reciprocal, f32r cast, rank-1 PE broadcast matmul,
                    # final DVE scale) return staggered so no engine FIFO
                    # backs up. `eng` alternates the inline copies between
                    # ScalarE and DVE per head.
                    den = small.tile([1, 512], f32, tag="den")
                    cu = work.tile([64, 512], f32r, tag="cu", bufs=3)
                    if eng == 0:
                        nc.scalar.activation(den[:], cps[64:65, :], Copy)
                        nc.scalar.activation(cu[:], cps[0:64, :], Copy)
                    else:
                        nc.vector.tensor_copy(den[:], cps[64:65, :])
                        nc.vector.tensor_copy(cu[:], cps[0:64, :])
                    r = small.tile([1, 512], f32, tag="r")
                    r2 = small.tile([1, 512], f32r, tag="r2")
                    rb = psum_pool.tile([65, 512], f32, tag="cps",
                                        bufs=cps_bufs, name=f"rb_{key}")
                    return [
                        (1, lambda: nc.vector.reciprocal_approx_fast(
                            r[:], den[:])),
                        (2, lambda: nc.vector.tensor_copy(r2[:], r[:])),
                        (3, lambda: nc.tensor.matmul(
                            rb[0:64, :], ones_col[:], r2[:],
                            start=True, stop=True)),
                        (4, lambda: nc.vector.tensor_mul(
                            ctx_dst, cu[:], rb[0:64, :])),
                    ]
                return normalize

            # ============ attention (heads 0,1 then head 2) ============
            # One continuous pool and flat 96-step loop so the pipeline
            # flows through the h01->h2 boundary without a PSUM pool
            # transition (a >2us PE gap there re-throttles the clock gate
            # for the whole tail otherwise).
            # Steps 0-63: heads 0,1 per (sq chunk c, sk chunk i) - two
            # score matmuls (h0 block A, h1 block B), ScalarE exp on A /
            # DVE fast-exp on B, two ctx accumulations. Steps 64-95:
            # head 2 per (sq-pair g, sk chunk i) with chunks 2g/2g+1 as
            # blocks A/B sharing one stationary kt. Pipelined 2 deep.
            with tc.tile_pool(name="pat", bufs=2, space="PSUM") as pat:
                normalize = make_normalize(pat, 3)
                ets, cps, cps2 = {}, {}, {}
                pending = {}

                def push_pieces(t, pieces):
                    for off, fn in pieces:
                        pending.setdefault(t + off, []).append(fn)

                def run_pieces(t):
                    for fn in pending.pop(t, ()):
                        fn()

                def sp_tile(name):
                    return pat.tile([P, 512], f32, tag="sp", bufs=4,
                                    name=name)

                def dummy_mms(sp, n):
                    # HAM insurance: overwritten by the next real matmul
                    for _ in range(n):
                        nc.tensor.matmul(sp[0:64, 0:128], warmw[:, :],
                                         x_sb[:, 0:128],
                                         start=True, stop=True)

                def exp_pair(t, sp_a, sp_b):
                    et_a = work.tile([P, 512], bf16, tag="et", bufs=6,
                                     name=f"etA_{t}")
                    nc.scalar.activation(et_a[:], sp_a[:], Exp, scale=SCALE)
                    et_b = work.tile([P, 512], bf16, tag="et", bufs=6,
                                     name=f"etB_{t}")
                    nc.vector.tensor_scalar(et_b[:].bitcast(i16), sp_b[:],
                                            FEXP_A, FEXP_B, MUL, ADD)
                    ets[t] = (et_a, et_b)

                def emit_scores(t):
                    if t < 64:
                        c, i = divmod(t, SKC)
                        sp_a = sp_tile(f"spA_{t}")
                        if i == 0:
                            dummy_mms(sp_a, 2)
                        nc.tensor.matmul(
                            sp_a[:], Bt[0:64, i * P:(i + 1) * P],
                            At[0:64, c * 512:(c + 1) * 512],
                            start=True, stop=True)
                        sp_b = sp_tile(f"spB_{t}")
                        nc.tensor.matmul(
                            sp_b[:], Ct[64:P, i * P:(i + 1) * P],
                            At[64:P, c * 512:(c + 1) * 512],
                            start=True, stop=True)
                        exp_pair(t, sp_a, sp_b)
                    else:
                        g, i = divmod(t - 64, SKC)
                        pair = []
                        for j in range(2):
                            c = 2 * g + j
                            sp = sp_tile(f"sp2_{t}_{j}")
                            if i == 0 and j == 0:
                                dummy_mms(sp, 2)
                            nc.tensor.matmul(
                                sp[:], Ct[0:64, i * P:(i + 1) * P],
                                qh2[:, c * 512:(c + 1) * 512],
                                start=True, stop=True)
                            pair.append(sp)
                        exp_pair(t, pair[0], pair[1])

                def emit_ctx(t):
                    et_a, et_b = ets.pop(t)
                    if t < 64:
                        c, i = divmod(t, SKC)
                        if i == 0:
                            cps[c] = [pat.tile([65, 512], f32, tag="cps",
                                               bufs=3, name=f"cps01_{c}_{j}")
                                      for j in range(2)]
                        nc.tensor.matmul(cps[c][0][:], v_sb[i][:, 0, :],
                                         et_a[:], start=(i == 0),
                                         stop=(i == SKC - 1))
                        nc.tensor.matmul(cps[c][1][:], v_sb[i][:, 1, :],
                                         et_b[:], start=(i == 0),
                                         stop=(i == SKC - 1))
                        if i == SKC - 1:
                            for j in range(2):
                                push_pieces(t + 1 + 2 * j, normalize(
                                    cps[c][j],
                                    ctx_a[j * 64:(j + 1) * 64,
                                          c * 512:(c + 1) * 512],
                                    f"a{c}_{j}", j))
                    else:
                        g, i = divmod(t - 64, SKC)
                        if i == 0:
                            cps2[g] = [pat.tile([65, 512], f32, tag="cps",
                                                bufs=3, name=f"cps2_{g}_{j}")
                                       for j in range(2)]
                        for j in range(2):
                            nc.tensor.matmul(
                                cps2[g][j][:], v_sb[i][:, 2, :],
                                (et_a, et_b)[j][:],
                                start=(i == 0), stop=(i == SKC - 1))
                        if i == SKC - 1:
                            for j in range(2):
                                push_pieces(t + 1 + 2 * j, normalize(
                                    cps2[g][j],
                                    ctx_b[:, (2 * g + j) * 512:
                                          (2 * g + j + 1) * 512],
                                    f"b{g}_{j}", j))

                def emit_outproj(c, e, op_tag):
                    op = pat.tile([P, 512], f32, tag=op_tag,
                                  bufs=(1 if op_tag == "op" else 4),
                                  name=f"op_{c}_{e}")
                    nc.tensor.matmul(
                        op[:], wo_sb[:, e * P:(e + 1) * P],
                        ctx_a[:, c * 512:(c + 1) * 512],
                        start=True, stop=False)
                    nc.tensor.matmul(
                        op[:], wo_sb[0:64, D + e * P: D + (e + 1) * P],
                        ctx_b[:, c * 512:(c + 1) * 512],
                        start=False, stop=True)
                    o = work.tile([P, 512], f32, tag="o", bufs=4,
                                  name=f"o_{c}_{e}")
                    if e % 2 == 0:
                        nc.vector.tensor_copy(o[:], op[:])
                    else:
                        nc.scalar.activation(o[:], op[:], Copy)
                    nc.sync.dma_start(
                        out=outT_d[e][:, c * 512:(c + 1) * 512], in_=o[:])

                # weave pair 0's output projection into pair 1 (steps
                # 80-95), starting at i=5 so the staggered normalize
                # multiplies land first; chunk 1's e=5 joins the tail
                weave_slots = list(range(5, SKC))
                DEPTH = 2
                NT = 96
                for t in range(NT + DEPTH + 8):
                    if t < NT:
                        emit_scores(t)
                    if DEPTH <= t < NT + DEPTH:
                        tt = t - DEPTH
                        emit_ctx(tt)
                        if tt >= 64:
                            g, i = divmod(tt - 64, SKC)
                            if g == 1 and i in weave_slots:
                                k = weave_slots.index(i)
                                emit_outproj(k // KD, k % KD, "op")
                    run_pieces(t - DEPTH)
                # tail: chunk 1 e=5 plus the final pair, on freed sp slots
                emit_outproj(1, 5, "sp")
                for c in (2, 3):
                    for e in range(KD):
                        emit_outproj(c, e, "sp")

    nc.finalize()
    return nc


def _get_nc():
    if "nc" not in _NC_CACHE:
        _NC_CACHE["nc"] = _build_bass()
    return _NC_CACHE["nc"]


def _core_inputs(c, x, w_q, b_q, w_k, w_v, w_o):
    import ml_dtypes
    bf = ml_dtypes.bfloat16
    b, g = divmod(c, 4)
    gs = slice(g * E, (g + 1) * E)

    def pack6(a):  # [768, W] -> [128, 6*W] chunk-packed
        w = a.shape[1]
        return np.ascontiguousarray(
            a.reshape(KD, P, w).transpose(1, 0, 2).reshape(P, KD * w))

    wqT = np.ascontiguousarray(w_q[gs, :].T)  # [768, 192]
    wkT = np.ascontiguousarray(w_k[gs, :].T)
    # m-tiles: [q0|q1], [k0|q2], [k2|k1]
    wqk = np.concatenate([wqT[:, 0:128], wkT[:, 0:64], wqT[:, 128:192],
                          wkT[:, 128:192], wkT[:, 64:128]], axis=1)
    bq = b_q[gs]
    bqp = np.zeros((1, 384), np.float32)
    bqp[0, 0:128] = bq[0:128]
    bqp[0, 192:256] = bq[128:192]
    woT = np.ascontiguousarray(w_o[:, gs].T)  # [192, 768]
    wop = np.zeros((P, 2 * D), np.float32)
    wop[:, 0:D] = woT[0:P]
    wop[0:64, D:] = woT[P:E]
    return {
        "xP": pack6(np.ascontiguousarray(x[b].T)).astype(bf),
        "wqkP": pack6(wqk).astype(bf),
        "wvP": pack6(np.ascontiguousarray(w_v[gs, :].T)).astype(bf),
        "woP": wop,
        "bqP": bqp.astype(bf),
    }


def kernel(x, w_q, b_q, w_k, b_k, w_v, b_v, w_o, b_o, _trace=False):
    from concourse.bass_utils import run_bass_kernel_spmd

    x = np.asarray(x, np.float32)
    w_q, b_q, w_k = (np.asarray(a, np.float32) for a in (w_q, b_q, w_k))
    w_v, w_o = np.asarray(w_v, np.float32), np.asarray(w_o, np.float32)
    b_v, b_o = np.asarray(b_v, np.float32), np.asarray(b_o, np.float32)

    nc = _get_nc()
    in_maps = [_core_inputs(c, x, w_q, b_q, w_k, w_v, w_o) for c in range(8)]
    res = run_bass_kernel_spmd(nc, in_maps, core_ids=list(range(8)),
                               trace=_trace)

    out = np.zeros((B, S, D), np.float32)
    for c in range(8):
        out[c // 4] += res.results[c]["outT"].T
    # K bias drops via softmax shift-invariance; V and O biases are constants
    out += b_o + b_v @ w_o.T
    if _trace:
        kernel._last_results = res
    return out
